# revision 1
# baseline (speedup 1.0000x reference)
"""Trainium2 Bass kernel for ClusteringMMD.

Per graph (batch-sharded 16+16 graphs onto each of 8 cores):
  - host pre-permutes the [512,512] adjacency to [128,4,512] so the
    device DMA is one fully contiguous 1MB transfer
  - ScalarE: one 2048-wide cast f32 -> fp8e4 (values are exactly 0/1)
  - TensorE: A^2 = A @ A via fp8 DoubleRow matmuls into PSUM (exact:
    0/1 products, fp32 accumulate); deg = ones^T @ A via two more
    DoubleRow matmuls (column-sum = row-sum for symmetric A)
  - VectorE: scalar_tensor_tensor fuses X = A^2 * A with accum_out =
    row-sum(X), yielding tri2 = diag(A^3) per node in one pass
  - DMA out tri2 [128,4] and deg [1,512] per graph
Host: bit-exact f32 replication of the reference's clustering-coefficient
binning (tri2/deg are exact small integers, so the device result is exact),
then the tiny [128,100] histogram MMD in f64.

The walrus build in this container rejects instructions carrying more than
one sync wait; _patch_compiler_wait_split() rewrites the BIR JSON right
before compilation, moving excess waits onto same-engine NoOps inserted
immediately before the over-subscribed instruction.
"""

import json
import numpy as np

B = 128
N = 512
BINS = 100
SIGMA = 1.0
N_CORES = 8
PER = B // N_CORES          # graphs per input tensor per core
GP = 2 * PER                # graphs per core (adj_1 shard + adj_2 shard)
P = 128
T = N // P                  # 4 row-blocks

MM_DTYPE = "fp8"            # "fp8" (DoubleRow) or "bf16"
WAIT_CAP = 1                # max sync waits this walrus accepts per inst

_NC_CACHE = {}


def _split_waits(bir_json, cap=WAIT_CAP):
    """Rewrite BIR JSON so no instruction carries more than `cap` sync
    waits; excess waits move to NoOps inserted just before it on the same
    engine (per-engine program order is list order within a block)."""
    m = json.loads(bir_json)
    ctr = 0
    for fn in m.get("functions", []):
        for blk in fn.get("blocks", []):
            out = []
            changed = False
            for ins in blk.get("instructions", []):
                si = ins.get("sync_info")
                waits = (si or {}).get("on_wait") or []
                if len(waits) > cap:
                    changed = True
                    for i in range(0, len(waits) - cap, cap):
                        ctr += 1
                        out.append(
                            {
                                "debug": ins.get("debug", 0),
                                "engine": ins["engine"],
                                "ins": [],
                                "name": f"WSPLIT-{ctr}",
                                "opcode": "NoOp",
                                "outs": [],
                                "text_hint": "wait_split",
                                "sync_info": {
                                    "on_wait": waits[i : i + cap],
                                    "on_update": [],
                                },
                            }
                        )
                    si["on_wait"] = waits[len(waits) - cap :]
                out.append(ins)
            if changed:
                blk["instructions"] = out
    return json.dumps(m).encode()


def _patch_compiler_wait_split():
    import concourse.bass_utils as bu
    import concourse.bass2jax as b2j

    if getattr(bu, "_wait_split_patched", False):
        return
    orig = bu.compile_bir_kernel

    def wrapped(bir_json, tmpdir, neff_name="file.neff"):
        return orig(_split_waits(bir_json), tmpdir, neff_name)

    bu.compile_bir_kernel = wrapped
    b2j.compile_bir_kernel = wrapped
    bu._wait_split_patched = True




def build_nc(gp=GP, mm_dtype=MM_DTYPE):
    import concourse.bass as bass
    import concourse.mybir as mybir
    from concourse.tile import TileContext
    from contextlib import ExitStack

    _patch_compiler_wait_split()
    dt = mybir.dt
    fp8 = mm_dtype == "fp8"
    cast_dt = dt.float8e4 if fp8 else dt.bfloat16

    nc = bass.Bass(
        "TRN2", target_bir_lowering=False, debug=False, num_devices=N_CORES
    )
    # input pre-permuted on host: a[g, p, t, n] = A_g[t*128 + p, n]
    a = nc.declare_dram_parameter("a", [gp, P, T, N], dt.float32, isOutput=False)
    # tri2 partition-major: ot[p, g*T + m] = tri2_g[m*128 + p]
    ot = nc.declare_dram_parameter("ot", [P, gp * T], dt.float32, isOutput=True)
    od = nc.declare_dram_parameter("od", [gp * N], dt.float32, isOutput=True)

    with TileContext(nc) as tc, ExitStack() as ctx:
        pconst = ctx.enter_context(tc.tile_pool(name="const", bufs=1))
        paf = ctx.enter_context(tc.tile_pool(name="af", bufs=8))
        pa8 = ctx.enter_context(tc.tile_pool(name="a8", bufs=4))
        px = ctx.enter_context(tc.tile_pool(name="xs", bufs=2))
        pps = ctx.enter_context(tc.tile_pool(name="ps", bufs=6, space="PSUM"))
        pdg = ctx.enter_context(tc.tile_pool(name="dg", bufs=2, space="PSUM"))

        # all-ones stationary operand for the deg column-sum matmuls;
        # [128, 2, 16] so the DoubleRow Ko-pair step is 16 bytes
        ones8 = pconst.tile([P, 2, 16], cast_dt)
        nc.vector.memset(ones8[:], 1.0)
        # whole-core result staging, one output DMA each at the end
        st_all = pconst.tile([P, gp * T], dt.float32)
        dg_all = pconst.tile([1, gp * N], dt.float32)

        for g in range(gp):
            af = paf.tile([P, T, N], dt.float32)
            nc.sync.dma_start(out=af[:], in_=a[g])
            a8 = pa8.tile([P, T, N], cast_dt)
            nc.scalar.activation(
                a8[:, :, :],
                af[:, :, :],
                mybir.ActivationFunctionType.Copy,
            )
            # deg = column-sum(A) (= row-sum, A symmetric) on the PE
            dg = pdg.tile([1, N], dt.float32)
            if fp8:
                for kk in range(T // 2):
                    nc.tensor.matmul(
                        dg[:],
                        ones8[:, :, 0:1],
                        a8[:, 2 * kk : 2 * kk + 2, :],
                        start=(kk == 0),
                        stop=(kk == T // 2 - 1),
                        perf_mode=mybir.MatmulPerfMode.DoubleRow,
                    )
            else:
                for k in range(T):
                    nc.tensor.matmul(
                        dg[:],
                        ones8[:, 0, 0:1],
                        a8[:, k, :],
                        start=(k == 0),
                        stop=(k == T - 1),
                    )
            for m in range(T):
                ps = pps.tile([P, N], dt.float32)
                if fp8:
                    for kk in range(T // 2):
                        nc.tensor.matmul(
                            ps[:],
                            a8[:, 2 * kk : 2 * kk + 2, m * P : (m + 1) * P],
                            a8[:, 2 * kk : 2 * kk + 2, :],
                            start=(kk == 0),
                            stop=(kk == T // 2 - 1),
                            perf_mode=mybir.MatmulPerfMode.DoubleRow,
                        )
                else:
                    for k in range(T):
                        nc.tensor.matmul(
                            ps[:],
                            a8[:, k, m * P : (m + 1) * P],
                            a8[:, k, :],
                            start=(k == 0),
                            stop=(k == T - 1),
                        )
                x = px.tile([P, N], dt.bfloat16)
                # X = (A^2 * 1.0) * A ; accum_out = rowsum(X) = tri2
                nc.vector.scalar_tensor_tensor(
                    x[:],
                    ps[:],
                    1.0,
                    af[:, m, :],
                    op0=mybir.AluOpType.mult,
                    op1=mybir.AluOpType.mult,
                    accum_out=st_all[:, g * T + m : g * T + m + 1],
                )
            nc.scalar.copy(dg_all[:, g * N : (g + 1) * N], dg[:])
            if (g + 1) % 8 == 0 or g == gp - 1:
                g0 = (g // 8) * 8
                # SWDGE so these don't head-of-line block the input queues
                nc.gpsimd.dma_start(
                    out=ot[:, g0 * T : (g + 1) * T],
                    in_=st_all[:, g0 * T : (g + 1) * T],
                )
                nc.gpsimd.dma_start(
                    out=od[g0 * N : (g + 1) * N].rearrange(
                        "(o f) -> o f", o=1
                    ),
                    in_=dg_all[:, g0 * N : (g + 1) * N],
                )
    return nc


def _get_nc():
    key = (GP, MM_DTYPE)
    if key not in _NC_CACHE:
        _NC_CACHE[key] = build_nc(*key)
    return _NC_CACHE[key]


def _permute_shard(shard):
    # [gp, 512, 512] -> [gp, 128, 4, 512] with [g, p, t, n] = A[g, t*128+p, n]
    gp = shard.shape[0]
    return np.ascontiguousarray(
        shard.reshape(gp, T, P, N).transpose(0, 2, 1, 3), dtype=np.float32
    )


def run_device(adj_1, adj_2, trace=False):
    """Run the bass kernel on 8 cores; returns (tri2, deg) for each input
    tensor as [B, N] f32 arrays, plus the BassKernelResults."""
    from concourse.bass_utils import run_bass_kernel_spmd

    nc = _get_nc()
    in_maps = []
    for c in range(N_CORES):
        shard = np.concatenate(
            [adj_1[c * PER : (c + 1) * PER], adj_2[c * PER : (c + 1) * PER]],
            axis=0,
        )
        in_maps.append({"a": _permute_shard(shard)})
    res = run_bass_kernel_spmd(nc, in_maps, list(range(N_CORES)), trace=trace)
    # ot [128, gp*4]: node m*128+p of graph g at ot[p, g*4+m]
    tri = np.stack(
        [
            r["ot"].reshape(P, GP, T).transpose(1, 2, 0).reshape(GP, N)
            for r in res.results
        ]
    )
    deg = np.stack([r["od"].reshape(GP, N) for r in res.results])
    tri2_1 = tri[:, :PER].reshape(B, N)
    tri2_2 = tri[:, PER:].reshape(B, N)
    deg_1 = deg[:, :PER].reshape(B, N)
    deg_2 = deg[:, PER:].reshape(B, N)
    return (tri2_1, deg_1), (tri2_2, deg_2), res


def _hist(tri2, deg):
    # bit-exact f32 replication of the reference binning
    tri2 = tri2.astype(np.float32)
    deg = deg.astype(np.float32)
    denom = deg * (deg - np.float32(1.0))
    c = np.where(
        denom > 0,
        tri2 / np.maximum(denom, np.float32(1.0)),
        np.float32(0.0),
    ).astype(np.float32)
    idx = np.clip((c * np.float32(BINS)).astype(np.int32), 0, BINS - 1)
    hist = np.zeros((idx.shape[0], BINS), np.float32)
    np.add.at(hist, (np.arange(idx.shape[0])[:, None], idx), np.float32(1.0))
    return hist


def _mmd(x, y):
    x = x.astype(np.float64)
    y = y.astype(np.float64)

    def kmat(a, b):
        sq = (
            (a * a).sum(-1)[:, None]
            + (b * b).sum(-1)[None, :]
            - 2.0 * (a @ b.T)
        )
        return np.exp(-np.maximum(sq, 0.0) / (2.0 * SIGMA * SIGMA))

    return kmat(x, x).mean() + kmat(y, y).mean() - 2.0 * kmat(x, y).mean()


def kernel(adj_1, adj_2):
    (t1, d1), (t2, d2), _ = run_device(adj_1, adj_2)
    h1 = _hist(t1, d1)
    h2 = _hist(t2, d2)
    return np.float32(_mmd(h1, h2))



# revision 9
# speedup vs baseline: 1.4437x; 1.4437x over previous
"""Trainium2 Bass kernel for ClusteringMMD.

Key algebraic trick: the host uploads B = fp8(A + 64*I) (entries {0,1,64},
all exact in fp8e4). The PE computes U = B @ B = A^2 + 128*A + 4096*I via
fp8 DoubleRow matmuls (exact integer arithmetic, f32 PSUM accumulate).
Since every off-diagonal A^2 entry is a common-neighbor count < 64:
    relu(U_ij - 64) = (A^2_ij + 64) * A_ij   for i != j
    relu(U_ii - 64) = deg_i + 4032
so  sum_j relu(U_ij - 64) = tri2_i + 65*deg_i + 4032
with tri2_i = diag(A^3)_i. This removes the elementwise multiply by A
entirely: the consumer per 128-row block is a SINGLE-INPUT relu+sum,
which both ScalarE (activation Relu, bias=-64, accum_out) and VectorE
(scalar_tensor_tensor max(ps-64, zeros), accum_out) can do in one
instruction per block, reading PSUM directly. Blocks are round-robined
between the two engines so they together keep up with the PE, which runs
nothing but the 8 DoubleRow matmuls per graph (its fp8 roofline).

Host: deg = rowsum(A) (trivial O(N^2)), tri2 = accum - 65*deg - 4032
(exact small integers in f32), then the reference's bit-exact f32
binning and the tiny [128,100] histogram MMD in f64.

The walrus build in this container rejects instructions carrying more than
one sync wait; _patch_compiler_wait_split() rewrites the BIR JSON right
before compilation, moving excess waits onto same-engine NoOps inserted
immediately before the over-subscribed instruction.
"""

import json
import numpy as np

B = 128
N = 512
BINS = 100
SIGMA = 1.0
N_CORES = 8
PER = B // N_CORES          # graphs per input tensor per core
GP = 2 * PER                # graphs per core (adj_1 shard + adj_2 shard)
P = 128
T = N // P                  # 4 row-blocks

CDIAG = 64.0                # diagonal boost; needs max(A^2 off-diag) < 64
# consumer route per block index (cycled): A=ScalarE relu-accum,
# D=VectorE relu-accum (both read the PSUM block directly)
ROUTE_PATTERN = "AD"
WAIT_CAP = 1                # max sync waits this walrus accepts per inst

_NC_CACHE = {}


def _split_waits(bir_json, cap=WAIT_CAP):
    """Rewrite BIR JSON so no instruction carries more than `cap` sync
    waits; excess waits move to NoOps inserted just before it on the same
    engine (per-engine program order is list order within a block)."""
    m = json.loads(bir_json)
    ctr = 0
    for fn in m.get("functions", []):
        for blk in fn.get("blocks", []):
            out = []
            changed = False
            for ins in blk.get("instructions", []):
                si = ins.get("sync_info")
                waits = (si or {}).get("on_wait") or []
                if len(waits) > cap:
                    changed = True
                    for i in range(0, len(waits) - cap, cap):
                        ctr += 1
                        out.append(
                            {
                                "debug": ins.get("debug", 0),
                                "engine": ins["engine"],
                                "ins": [],
                                "name": f"WSPLIT-{ctr}",
                                "opcode": "NoOp",
                                "outs": [],
                                "text_hint": "wait_split",
                                "sync_info": {
                                    "on_wait": waits[i : i + cap],
                                    "on_update": [],
                                },
                            }
                        )
                    si["on_wait"] = waits[len(waits) - cap :]
                out.append(ins)
            if changed:
                blk["instructions"] = out
    return json.dumps(m).encode()


def _patch_compiler_wait_split():
    import concourse.bass_utils as bu
    import concourse.bass2jax as b2j

    if getattr(bu, "_wait_split_patched", False):
        return
    orig = bu.compile_bir_kernel

    def wrapped(bir_json, tmpdir, neff_name="file.neff"):
        return orig(_split_waits(bir_json), tmpdir, neff_name)

    bu.compile_bir_kernel = wrapped
    b2j.compile_bir_kernel = wrapped
    bu._wait_split_patched = True


def _routes(gp=GP):
    """Route letter for each of the gp*T consumer blocks."""
    pat = ROUTE_PATTERN
    return [pat[i % len(pat)] for i in range(gp * T)]


def build_nc(gp=GP):
    import concourse.bass as bass
    import concourse.mybir as mybir
    from concourse.tile import TileContext
    from contextlib import ExitStack

    _patch_compiler_wait_split()
    dt = mybir.dt

    nc = bass.Bass(
        "TRN2", target_bir_lowering=False, debug=False, num_devices=N_CORES
    )
    # input pre-permuted+pre-cast on host:
    #   a[g, p, t, n] = fp8((A_g + 64*I)[t*128 + p, n])
    a = nc.declare_dram_parameter("a", [gp, P, T, N], dt.float8e4, isOutput=False)

    routes = _routes(gp)
    counts = {r: routes.count(r) for r in "AD"}
    # per-route accum outputs, partition-major: o[p, k] = accum of node
    # (m*128+p) of the k-th block routed to that engine
    outs = {}
    for r in "AD":
        if counts[r]:
            outs[r] = nc.declare_dram_parameter(
                f"o{r}", [P, counts[r]], dt.float32, isOutput=True
            )

    with TileContext(nc) as tc, ExitStack() as ctx:
        pconst = ctx.enter_context(tc.tile_pool(name="const", bufs=1))
        pa8 = ctx.enter_context(tc.tile_pool(name="a8", bufs=4))
        pps = ctx.enter_context(tc.tile_pool(name="ps", bufs=7, space="PSUM"))
        pxa = ctx.enter_context(tc.tile_pool(name="xa", bufs=2))
        pxd = ctx.enter_context(tc.tile_pool(name="xd", bufs=2))

        zeros = pconst.tile([P, N], dt.bfloat16, name="zeros")
        nc.vector.memset(zeros[:], 0.0)
        nbias = pconst.tile([P, 1], dt.float32, name="nbias")
        nc.vector.memset(nbias[:], -CDIAG)
        # whole-core result staging, one output DMA per route at the end
        st = {
            r: pconst.tile([P, counts[r]], dt.float32, name=f"st{r}")
            for r in "AD"
            if counts[r]
        }
        slot = {r: 0 for r in "AD"}

        bi = 0
        for g in range(gp):
            a8 = pa8.tile([P, T, N], dt.float8e4)
            nc.sync.dma_start(out=a8[:], in_=a[g])
            for m in range(T):
                ps = pps.tile([P, N], dt.float32)
                for kk in range(T // 2):
                    nc.tensor.matmul(
                        ps[:],
                        a8[:, 2 * kk : 2 * kk + 2, m * P : (m + 1) * P],
                        a8[:, 2 * kk : 2 * kk + 2, :],
                        start=(kk == 0),
                        stop=(kk == T // 2 - 1),
                        perf_mode=mybir.MatmulPerfMode.DoubleRow,
                    )
                r = routes[bi]
                k = slot[r]
                slot[r] += 1
                acc = st[r][:, k : k + 1]
                if r == "A":
                    xa = pxa.tile([P, N], dt.bfloat16)
                    nc.scalar.activation(
                        xa[:],
                        ps[:],
                        mybir.ActivationFunctionType.Relu,
                        bias=nbias[:],
                        scale=1.0,
                        accum_out=acc,
                    )
                else:  # "D"
                    xd = pxd.tile([P, N], dt.bfloat16)
                    nc.vector.scalar_tensor_tensor(
                        xd[:],
                        ps[:],
                        CDIAG,
                        zeros[:],
                        op0=mybir.AluOpType.subtract,
                        op1=mybir.AluOpType.max,
                        accum_out=acc,
                    )
                bi += 1
        for r in "AD":
            if counts[r]:
                nc.gpsimd.dma_start(out=outs[r][:], in_=st[r][:])
    return nc


def _get_nc():
    key = (GP, ROUTE_PATTERN)
    if key not in _NC_CACHE:
        _NC_CACHE[key] = build_nc(GP)
    return _NC_CACHE[key]


_FP8_ONE = np.uint8(0x38)   # 1.0 in float8_e4m3
_FP8_64 = np.uint8(0x68)    # 64.0 in float8_e4m3


def _permute_shard_fp8(shard):
    # [gp, 512, 512] f32 -> [gp, 128, 4, 512] fp8 with
    # [g, p, t, n] = fp8((A + 64*I)[g, t*128+p, n])
    import ml_dtypes

    gp = shard.shape[0]
    u8 = (shard != 0).astype(np.uint8) * _FP8_ONE
    ii = np.arange(N)
    u8[:, ii, ii] = _FP8_64
    u8 = np.ascontiguousarray(u8.reshape(gp, T, P, N).transpose(0, 2, 1, 3))
    return u8.view(ml_dtypes.float8_e4m3)


def run_device(adj_1, adj_2, trace=False):
    """Run the bass kernel on 8 cores; returns the per-node relu-accum
    (= tri2 + 65*deg + 4032) for each input tensor as [B, N] f32 arrays,
    plus the BassKernelResults."""
    from concourse.bass_utils import run_bass_kernel_spmd

    nc = _get_nc()
    in_maps = []
    for c in range(N_CORES):
        shard = np.concatenate(
            [adj_1[c * PER : (c + 1) * PER], adj_2[c * PER : (c + 1) * PER]],
            axis=0,
        )
        in_maps.append({"a": _permute_shard_fp8(shard)})
    res = run_bass_kernel_spmd(nc, in_maps, list(range(N_CORES)), trace=trace)

    routes = _routes(GP)
    acc = np.empty((N_CORES, GP, T, P), np.float32)
    for c, r in enumerate(res.results):
        slot = {q: 0 for q in "AD"}
        for bi, q in enumerate(routes):
            g, m = divmod(bi, T)
            acc[c, g, m, :] = r[f"o{q}"][:, slot[q]]
            slot[q] += 1
    acc = acc.reshape(N_CORES, GP, N)
    acc_1 = acc[:, :PER].reshape(B, N)
    acc_2 = acc[:, PER:].reshape(B, N)
    return acc_1, acc_2, res


def _tri2_from_acc(acc, deg):
    # acc = tri2 + 65*deg + 4032 (all exact small integers in f32)
    return acc - np.float32(65.0) * deg - np.float32(CDIAG * CDIAG - CDIAG)


def _hist(tri2, deg):
    # bit-exact f32 replication of the reference binning
    tri2 = tri2.astype(np.float32)
    deg = deg.astype(np.float32)
    denom = deg * (deg - np.float32(1.0))
    c = np.where(
        denom > 0,
        tri2 / np.maximum(denom, np.float32(1.0)),
        np.float32(0.0),
    ).astype(np.float32)
    idx = np.clip((c * np.float32(BINS)).astype(np.int32), 0, BINS - 1)
    hist = np.zeros((idx.shape[0], BINS), np.float32)
    np.add.at(hist, (np.arange(idx.shape[0])[:, None], idx), np.float32(1.0))
    return hist


def _mmd(x, y):
    x = x.astype(np.float64)
    y = y.astype(np.float64)

    def kmat(a, b):
        sq = (
            (a * a).sum(-1)[:, None]
            + (b * b).sum(-1)[None, :]
            - 2.0 * (a @ b.T)
        )
        return np.exp(-np.maximum(sq, 0.0) / (2.0 * SIGMA * SIGMA))

    return kmat(x, x).mean() + kmat(y, y).mean() - 2.0 * kmat(x, y).mean()


def _tri2_host(adj):
    a2 = np.matmul(adj, adj)
    return np.einsum("bij,bji->bi", a2, adj).astype(np.float32)


def kernel(adj_1, adj_2):
    # deg is a trivial O(N^2) rowsum; exact small integers in f32
    d1 = adj_1.sum(-1, dtype=np.float32)
    d2 = adj_2.sum(-1, dtype=np.float32)
    if max(d1.max(), d2.max()) >= CDIAG:
        # relu threshold trick needs max common-neighbor count < 64;
        # essentially impossible for this input distribution, but stay
        # correct on arbitrary inputs
        t1, t2 = _tri2_host(adj_1), _tri2_host(adj_2)
    else:
        a1, a2, _ = run_device(adj_1, adj_2)
        t1 = _tri2_from_acc(a1, d1)
        t2 = _tri2_from_acc(a2, d2)
    h1 = _hist(t1, d1)
    h2 = _hist(t2, d2)
    return np.float32(_mmd(h1, h2))


# revision 11
# speedup vs baseline: 1.5951x; 1.1049x over previous
"""Trainium2 Bass kernel for ClusteringMMD.

Key algebraic trick: the host uploads B = fp8(A + 64*I) (entries {0,1,64},
all exact in fp8e4). The PE computes U = B @ B = A^2 + 128*A + 4096*I via
fp8 DoubleRow matmuls (exact integer arithmetic, f32 PSUM accumulate).
Since every off-diagonal A^2 entry is a common-neighbor count < 64:
    relu(U_ij - 64) = (A^2_ij + 64) * A_ij   for i != j
    relu(U_ii - 64) = deg_i + 4032
so  sum_j relu(U_ij - 64) = tri2_i + 65*deg_i + 4032
with tri2_i = diag(A^3)_i. This removes the elementwise multiply by A
entirely: the consumer per 128-row block is a SINGLE-INPUT relu+sum,
which both ScalarE (activation Relu, bias=-64, accum_out) and VectorE
(scalar_tensor_tensor max(ps-64, zeros), accum_out) can do in one
instruction per block, reading PSUM directly. Blocks are round-robined
between the two engines so they together keep up with the PE, which runs
nothing but the 8 DoubleRow matmuls per graph (its fp8 roofline).

Host: deg = rowsum(A) (trivial O(N^2)), tri2 = accum - 65*deg - 4032
(exact small integers in f32), then the reference's bit-exact f32
binning and the tiny [128,100] histogram MMD in f64.

The walrus build in this container rejects instructions carrying more than
one sync wait; _patch_compiler_wait_split() rewrites the BIR JSON right
before compilation, moving excess waits onto same-engine NoOps inserted
immediately before the over-subscribed instruction.
"""

import json
import numpy as np

B = 128
N = 512
BINS = 100
SIGMA = 1.0
N_CORES = 8
PER = B // N_CORES          # graphs per input tensor per core
GP = 2 * PER                # graphs per core (adj_1 shard + adj_2 shard)
P = 128
T = N // P                  # 4 row-blocks

CDIAG = 64.0                # diagonal boost; needs max(A^2 off-diag) < 64
# consumer route per block index (cycled): A=ScalarE relu-accum,
# D=VectorE relu-accum (both read the PSUM block directly)
ROUTE_PATTERN = "AD"
WAIT_CAP = 1                # max sync waits this walrus accepts per inst

_NC_CACHE = {}


def _split_waits(bir_json, cap=WAIT_CAP):
    """Rewrite BIR JSON so no instruction carries more than `cap` sync
    waits; excess waits move to NoOps inserted just before it on the same
    engine (per-engine program order is list order within a block)."""
    m = json.loads(bir_json)
    ctr = 0
    for fn in m.get("functions", []):
        for blk in fn.get("blocks", []):
            out = []
            changed = False
            for ins in blk.get("instructions", []):
                si = ins.get("sync_info")
                waits = (si or {}).get("on_wait") or []
                if len(waits) > cap:
                    changed = True
                    for i in range(0, len(waits) - cap, cap):
                        ctr += 1
                        out.append(
                            {
                                "debug": ins.get("debug", 0),
                                "engine": ins["engine"],
                                "ins": [],
                                "name": f"WSPLIT-{ctr}",
                                "opcode": "NoOp",
                                "outs": [],
                                "text_hint": "wait_split",
                                "sync_info": {
                                    "on_wait": waits[i : i + cap],
                                    "on_update": [],
                                },
                            }
                        )
                    si["on_wait"] = waits[len(waits) - cap :]
                out.append(ins)
            if changed:
                blk["instructions"] = out
    return json.dumps(m).encode()


def _patch_compiler_wait_split():
    import concourse.bass_utils as bu
    import concourse.bass2jax as b2j

    if getattr(bu, "_wait_split_patched", False):
        return
    orig = bu.compile_bir_kernel

    def wrapped(bir_json, tmpdir, neff_name="file.neff"):
        return orig(_split_waits(bir_json), tmpdir, neff_name)

    bu.compile_bir_kernel = wrapped
    b2j.compile_bir_kernel = wrapped
    bu._wait_split_patched = True


def _routes(gp=GP):
    """Route letter for each of the gp*T consumer blocks."""
    pat = ROUTE_PATTERN
    return [pat[i % len(pat)] for i in range(gp * T)]


def build_nc(gp=GP):
    import concourse.bass as bass
    import concourse.mybir as mybir
    from concourse.tile import TileContext
    from contextlib import ExitStack

    _patch_compiler_wait_split()
    dt = mybir.dt

    nc = bass.Bass(
        "TRN2", target_bir_lowering=False, debug=False, num_devices=N_CORES
    )
    # input pre-permuted+pre-cast on host:
    #   a[g, p, t, n] = fp8((A_g + 64*I)[t*128 + p, n])
    a = nc.declare_dram_parameter("a", [gp, P, T, N], dt.float8e4, isOutput=False)

    routes = _routes(gp)
    counts = {r: routes.count(r) for r in "AD"}
    # per-route accum outputs, partition-major: o[p, k] = accum of node
    # (m*128+p) of the k-th block routed to that engine
    outs = {}
    for r in "AD":
        if counts[r]:
            outs[r] = nc.declare_dram_parameter(
                f"o{r}", [P, counts[r]], dt.float32, isOutput=True
            )

    with TileContext(nc) as tc, ExitStack() as ctx:
        pconst = ctx.enter_context(tc.tile_pool(name="const", bufs=1))
        pa8 = ctx.enter_context(tc.tile_pool(name="a8", bufs=4))
        pps = ctx.enter_context(tc.tile_pool(name="ps", bufs=7, space="PSUM"))
        pxa = ctx.enter_context(tc.tile_pool(name="xa", bufs=2))
        pxd = ctx.enter_context(tc.tile_pool(name="xd", bufs=2))

        zeros = pconst.tile([P, N], dt.bfloat16, name="zeros")
        nc.vector.memset(zeros[:], 0.0)
        nbias = pconst.tile([P, 1], dt.float32, name="nbias")
        nc.vector.memset(nbias[:], -CDIAG)
        # whole-core result staging, one output DMA per route at the end
        st = {
            r: pconst.tile([P, counts[r]], dt.float32, name=f"st{r}")
            for r in "AD"
            if counts[r]
        }
        slot = {r: 0 for r in "AD"}

        bi = 0
        for g in range(gp):
            a8 = pa8.tile([P, T, N], dt.float8e4)
            nc.sync.dma_start(out=a8[:], in_=a[g])
            for m in range(T):
                ps = pps.tile([P, N], dt.float32)
                for kk in range(T // 2):
                    nc.tensor.matmul(
                        ps[:],
                        a8[:, 2 * kk : 2 * kk + 2, m * P : (m + 1) * P],
                        a8[:, 2 * kk : 2 * kk + 2, :],
                        start=(kk == 0),
                        stop=(kk == T // 2 - 1),
                        perf_mode=mybir.MatmulPerfMode.DoubleRow,
                    )
                r = routes[bi]
                k = slot[r]
                slot[r] += 1
                acc = st[r][:, k : k + 1]
                if r == "A":
                    xa = pxa.tile([P, N], dt.bfloat16)
                    nc.scalar.activation(
                        xa[:],
                        ps[:],
                        mybir.ActivationFunctionType.Relu,
                        bias=nbias[:],
                        scale=1.0,
                        accum_out=acc,
                    )
                else:  # "D"
                    xd = pxd.tile([P, N], dt.bfloat16)
                    nc.vector.scalar_tensor_tensor(
                        xd[:],
                        ps[:],
                        CDIAG,
                        zeros[:],
                        op0=mybir.AluOpType.subtract,
                        op1=mybir.AluOpType.max,
                        accum_out=acc,
                    )
                bi += 1
        for r in "AD":
            if counts[r]:
                nc.gpsimd.dma_start(out=outs[r][:], in_=st[r][:])
    return nc


def _get_nc():
    key = (GP, ROUTE_PATTERN)
    if key not in _NC_CACHE:
        _NC_CACHE[key] = build_nc(GP)
    return _NC_CACHE[key]


_FP8_ONE = np.uint8(0x38)   # 1.0 in float8_e4m3
_FP8_64 = np.uint8(0x68)    # 64.0 in float8_e4m3


def _permute_shard_fp8(shard):
    # [gp, 512, 512] f32 -> [gp, 128, 4, 512] fp8 with
    # [g, p, t, n] = fp8((A + 64*I)[g, t*128+p, n])
    import ml_dtypes

    gp = shard.shape[0]
    u8 = (shard != 0).astype(np.uint8) * _FP8_ONE
    ii = np.arange(N)
    u8[:, ii, ii] = _FP8_64
    u8 = np.ascontiguousarray(u8.reshape(gp, T, P, N).transpose(0, 2, 1, 3))
    return u8.view(ml_dtypes.float8_e4m3)


def run_device(adj_1, adj_2, trace=False):
    """Run the bass kernel on 8 cores; returns the per-node relu-accum
    (= tri2 + 65*deg + 4032) for each input tensor as [B, N] f32 arrays,
    plus the BassKernelResults."""
    from concourse.bass_utils import run_bass_kernel_spmd

    nc = _get_nc()
    in_maps = []
    for c in range(N_CORES):
        shard = np.concatenate(
            [adj_1[c * PER : (c + 1) * PER], adj_2[c * PER : (c + 1) * PER]],
            axis=0,
        )
        in_maps.append({"a": _permute_shard_fp8(shard)})
    res = run_bass_kernel_spmd(nc, in_maps, list(range(N_CORES)), trace=trace)

    routes = _routes(GP)
    acc = np.empty((N_CORES, GP, T, P), np.float32)
    for c, r in enumerate(res.results):
        slot = {q: 0 for q in "AD"}
        for bi, q in enumerate(routes):
            g, m = divmod(bi, T)
            acc[c, g, m, :] = r[f"o{q}"][:, slot[q]]
            slot[q] += 1
    acc = acc.reshape(N_CORES, GP, N)
    acc_1 = acc[:, :PER].reshape(B, N)
    acc_2 = acc[:, PER:].reshape(B, N)
    return acc_1, acc_2, res


def _tri2_from_acc(acc, deg, pedge):
    # acc = tri2 + 65*deg + 4032 - A[i, i^128] (all exact small integers in
    # f32). The last term: the PE's DoubleRow pair-adder sums the k=i
    # product (64*64=4096) with the k=i^128 product (A[i^128, i]) in
    # reduced precision, and 4097 rounds to 4096 (RNE in fp16 or bf16
    # alike), deterministically dropping the partner-edge bit.
    return (
        acc
        - np.float32(65.0) * deg
        - np.float32(CDIAG * CDIAG - CDIAG)
        + pedge
    )


def _partner_edge(adj):
    # pedge[b, i] = A[b, i, i ^ 128]
    ii = np.arange(N)
    return adj[:, ii, ii ^ 128].astype(np.float32)


def _hist(tri2, deg):
    # bit-exact f32 replication of the reference binning
    tri2 = tri2.astype(np.float32)
    deg = deg.astype(np.float32)
    denom = deg * (deg - np.float32(1.0))
    c = np.where(
        denom > 0,
        tri2 / np.maximum(denom, np.float32(1.0)),
        np.float32(0.0),
    ).astype(np.float32)
    idx = np.clip((c * np.float32(BINS)).astype(np.int32), 0, BINS - 1)
    hist = np.zeros((idx.shape[0], BINS), np.float32)
    np.add.at(hist, (np.arange(idx.shape[0])[:, None], idx), np.float32(1.0))
    return hist


def _mmd(x, y):
    x = x.astype(np.float64)
    y = y.astype(np.float64)

    def kmat(a, b):
        sq = (
            (a * a).sum(-1)[:, None]
            + (b * b).sum(-1)[None, :]
            - 2.0 * (a @ b.T)
        )
        return np.exp(-np.maximum(sq, 0.0) / (2.0 * SIGMA * SIGMA))

    return kmat(x, x).mean() + kmat(y, y).mean() - 2.0 * kmat(x, y).mean()


def _tri2_host(adj):
    a2 = np.matmul(adj, adj)
    return np.einsum("bij,bji->bi", a2, adj).astype(np.float32)


def kernel(adj_1, adj_2):
    # deg is a trivial O(N^2) rowsum; exact small integers in f32
    d1 = adj_1.sum(-1, dtype=np.float32)
    d2 = adj_2.sum(-1, dtype=np.float32)
    if max(d1.max(), d2.max()) >= CDIAG:
        # relu threshold trick needs max common-neighbor count < 64;
        # essentially impossible for this input distribution, but stay
        # correct on arbitrary inputs
        t1, t2 = _tri2_host(adj_1), _tri2_host(adj_2)
    else:
        a1, a2, _ = run_device(adj_1, adj_2)
        t1 = _tri2_from_acc(a1, d1, _partner_edge(adj_1))
        t2 = _tri2_from_acc(a2, d2, _partner_edge(adj_2))
    h1 = _hist(t1, d1)
    h2 = _hist(t2, d2)
    return np.float32(_mmd(h1, h2))
